# revision 25
# baseline (speedup 1.0000x reference)
"""Trainium2 Bass kernel for nn_ConvFilter (geometric-series conv filter).

Math (per batch b, output position l, feature f):
    t[o,l]  = sum_{i,k} conv_w[o,i,k] * x[l+k,i]          (valid conv, L=S-K+1)
    tau     = sigmoid(t + bias)
    out     = (sum_i tau^(7-i) * x[l+i,f]) / (sum_i tau^i)

v2 design (bottleneck was DVE at 75% busy):
  * x converted to fp16 on host; two dram copies (x16 and x16 shifted by one
    seq element) are DMA'd straight to SBUF -> no on-device fp32->fp16 ACT
    passes, and the conv matmuls run on the same fp16 tiles (full-rate PE).
  * numerator in fp16 on DVE (2x packed mode):
        q_j = tau*x_{2j} + x_{2j+1}
        N   = (q0*T2 + q1)*T4 + (q2*T2 + q3),   T2 = tau^2, T4 = tau^4
    odd-shift windows read the one-element-shifted x copy so every window
    stays 4-byte aligned (keeps the DVE 2x mode).
  * denominator D = (1+tau)(1+tau^2)(1+tau^4) as one custom DVE op (fp16 out);
    reciprocal moved to the ACT engine (builtin Reciprocal table);
    out = N * rho in fp16, DMA'd out as fp16, host converts to fp32.
  * engine split: ACT does sigmoid/squares/reciprocal, DVE the 14-op tree +
    denom + final mult, PE only matmuls.
  * data-parallel over batch: 8 batches/core on 8 cores, weights replicated.
"""

import numpy as np
from contextlib import ExitStack

import concourse.bass as bass
import concourse.tile as tile
from concourse import bacc, mybir
from concourse.bass_utils import run_bass_kernel_spmd
from concourse import dve_ops
from concourse.dve_ops import DveOp
from concourse.dve_spec import Spec, Src0, Src1, lower, sq, One, _has_src1
from concourse.dve_uop import DveOpSpec

B, S, F, K = 64, 1024, 256, 8
L = S - K + 1  # 1017
NCORES = 8
BPC = B // NCORES
P = 128
NFB = F // P  # 2 feature blocks
LT = 512      # matmul l-tile width (one PSUM bank)
LE = L + 1    # even fp16 elementwise width (DVE 2x mode needs even counts)


def _register_op(name, spec, subdim=False):
    for existing in dve_ops.OPS:
        if existing.name == name:
            return existing
    shas = {}
    for ver in ("v3", "v4"):
        tmp = DveOpSpec(name=name, opcode=0, uops=lower(spec, ver=ver),
                        rd1_en=_has_src1(spec))
        shas[ver] = tmp.sha(ver)
    op = DveOp(name, spec, subdim=subdim, uops_sha=shas)
    dve_ops.OPS.append(op)
    dve_ops.CUSTOM_DVE_SPECS[name] = spec
    dve_ops._SUB_OPCODE_FOR_NAME[name] = (
        dve_ops._CUSTOM_DVE_ROW_BASE + len(dve_ops.OPS) - 1
    )
    assert dve_ops._SUB_OPCODE_FOR_NAME[name] < 0x20
    return op


def _get_ops():
    _t2 = sq(Src0)
    _t4 = sq(_t2)
    denom_spec = Spec(
        body=(Src0 + One) * (_t2 + One) * (_t4 + One),
        reference=lambda in0, in1, s0, s1, imm2: (
            (1.0 + in0) * (1.0 + in0 * in0) * (1.0 + in0 ** 4)
        ).astype(np.float32),
    )
    return _register_op("ANT_CF_DENOM", denom_spec)


def build_module():
    DENOM_OP = _get_ops()
    f32 = mybir.dt.float32
    f16 = mybir.dt.float16
    TT = mybir.AluOpType
    SIG = mybir.ActivationFunctionType.Sigmoid
    SQU = mybir.ActivationFunctionType.Square
    LN = mybir.ActivationFunctionType.Ln
    EXP = mybir.ActivationFunctionType.Exp

    nc = bacc.Bacc("TRN2", target_bir_lowering=False, debug=False,
                   enable_asserts=False, num_devices=NCORES)

    xh_d = nc.dram_tensor("xh", [BPC, NFB, P, S], f16, kind="ExternalInput").ap()
    xo_d = nc.dram_tensor("xo", [BPC, NFB, P, S], f16, kind="ExternalInput").ap()
    wt_d = nc.dram_tensor("wt", [P, K * NFB * F], f16, kind="ExternalInput").ap()
    cb_d = nc.dram_tensor("cb", [F, 1], f32, kind="ExternalInput").ap()
    yt_d = nc.dram_tensor("yt", [BPC, NFB, P, L], f16, kind="ExternalOutput").ap()

    def mkap(base, off, dims):
        # raw AP: keep the tile's partition dim, replace free dims;
        # dims = [(stride, size), ...] in elements
        return bass.AP(base.tensor, base.offset + off,
                       [base.ap[0]] + [[s, n] for s, n in dims])

    with tile.TileContext(nc) as tc, ExitStack() as ctx:
        wpool = ctx.enter_context(tc.tile_pool(name="w", bufs=1))
        xpool = ctx.enter_context(tc.tile_pool(name="x", bufs=2))
        tpool = ctx.enter_context(tc.tile_pool(name="t", bufs=3))
        qpool = ctx.enter_context(tc.tile_pool(name="q", bufs=1))
        opool = ctx.enter_context(tc.tile_pool(name="o", bufs=2))
        ppool = ctx.enter_context(tc.tile_pool(name="p", bufs=2, space="PSUM"))

        # weights + bias: loaded once; host pre-packs the [p, (k ic f)]
        # layout, and the load is split into 4 DMAs so it spreads across
        # hardware queues (one instruction rides a single ~22GB/s engine)
        w_all = wpool.tile([P, K * NFB * F], f16, tag="wall")
        WQ = K * NFB * F // 4
        for c in range(4):
            nc.sync.dma_start(w_all[:, c * WQ:(c + 1) * WQ],
                              wt_d[:, c * WQ:(c + 1) * WQ])

        def wslice(k, ic, ob):
            base = (k * NFB + ic) * F + ob * P
            return w_all[:, base:base + P]

        bias_sb = wpool.tile([P, NFB], f32, tag="bias")
        nc.sync.dma_start(
            bias_sb[:], cb_d.rearrange("(ob p) one -> p (ob one)", p=P))

        W2 = NFB * S

        # warm-up: ramp the PE clock while the weight/x DMAs land; reads a
        # never-written scratch tile (values irrelevant), results overwritten
        # by batch 0's first accumulation (start=True).
        warm = wpool.tile([P, LT], f16, tag="warm")
        nc.gpsimd.memset(warm[:], 0.0)

        for b in range(BPC):
            # fp16 x (even-aligned) and its one-element-shifted copy, both
            # feature blocks side by side: [128, 2048] fp16; one DMA each
            xh = xpool.tile([P, W2], f16, tag="xh")
            xho = xpool.tile([P, W2], f16, tag="xho")
            # b0: also split along seq at col 520 (= LT + K) so the first
            # conv half + tree half only wait for the first chunk
            chunks = ((0, LT + K), (LT + K, S)) if b == 0 else ((0, S),)
            for src_d, dst in ((xh_d, xh), (xo_d, xho)):
                for ic in range(NFB):  # per-FB: parallel queues
                    for c0, c1 in chunks:
                        nc.sync.dma_start(
                            dst[:, ic * S + c0: ic * S + c1],
                            bass.AP(src_d.tensor,
                                    (b * NFB + ic) * P * S + c0,
                                    [[S, P], [1, c1 - c0]]))

            pss = {}
            for ob in range(NFB):
                for li, l0 in enumerate((0, L - LT)):
                    pss[(ob, li)] = ppool.tile([P, LT], f32, tag=f"ps{ob}{li}",
                                               name=f"ps{ob}{li}_{b}")

            def conv_half(li, l0):
                # ob-major so sigmoid(ob=0) can fire after 16 MMs
                for ob in range(NFB):
                    for ic in range(NFB):
                        for k in range(K):
                            first = (ic == 0 and k == 0)
                            last = (ic == NFB - 1 and k == K - 1)
                            nc.tensor.matmul(
                                pss[(ob, li)][:],
                                wslice(k, ic, ob),
                                xh[:, ic * S + l0 + k: ic * S + l0 + k + LT],
                                start=first, stop=last,
                            )

            def conv_both():
                # weight-major so each LDWEIGHTS serves both l-tiles
                for ic in range(NFB):
                    for k in range(K):
                        first = (ic == 0 and k == 0)
                        last = (ic == NFB - 1 and k == K - 1)
                        for ob in range(NFB):
                            for li, l0 in enumerate((0, L - LT)):
                                nc.tensor.matmul(
                                    pss[(ob, li)][:],
                                    wslice(k, ic, ob),
                                    xh[:, ic * S + l0 + k:
                                       ic * S + l0 + k + LT],
                                    start=first, stop=last,
                                )

            tau = tpool.tile([P, W2], f16, tag="tau")
            t2 = tpool.tile([P, W2], f16, tag="t2")
            t4 = tpool.tile([P, W2], f16, tag="t4")
            uu = qpool.tile([P, 4 * W2], f16, tag="uu")
            qq = qpool.tile([P, 4 * W2], f16, tag="qq")
            mh = qpool.tile([P, 2 * W2], f16, tag="mh")
            hh = qpool.tile([P, 2 * W2], f16, tag="hh")
            m1 = qpool.tile([P, W2], f16, tag="m1")
            nh = qpool.tile([P, W2], f16, tag="nh")

            def sigmoid_half(li, l0):
                for ob in range(NFB):
                    nc.scalar.activation(
                        tau[:, ob * S + l0: ob * S + l0 + LT],
                        pss[(ob, li)][:], SIG,
                        bias=bias_sb[:, ob:ob + 1], scale=1.0)

            def squares(n0, nw, c0=0, cn=NFB):
                base = c0 * S + n0
                sub = [(S, cn), (1, nw)]
                nc.scalar.activation(mkap(t2[:], base, sub),
                                     mkap(tau[:], base, sub), SQU)
                nc.scalar.activation(mkap(t4[:], base, sub),
                                     mkap(t2[:], base, sub), SQU)

            def tree(n0, nw, c0=0, cn=NFB):
                # Merged numerator tree: independent ops fused along an extra
                # j dim (stride-2 windows on x, stride-0 broadcast on tau/t2).
                # u_j = tau * x_{2j}            j=0..3   (one op)
                # q_j = u_j + x_{2j+1}          j=0..3   (one op)
                # (m0, h1) = (q0, q2) * t2               (one op)
                # (h0, h2) = (m0, h1) + (q1, q3)         (one op)
                # m1 = h0 * t4 ; nh = m1 + h2            (two ops)
                b0_ = c0 * S + n0
                CN = [(S, cn), (1, nw)]
                nc.vector.tensor_tensor(
                    mkap(uu[:], b0_, [(W2, 4)] + CN),
                    mkap(tau[:], b0_, [(0, 4)] + CN),
                    mkap(xh[:], b0_, [(2, 4)] + CN), TT.mult)
                nc.vector.tensor_tensor(
                    mkap(qq[:], b0_, [(W2, 4)] + CN),
                    mkap(uu[:], b0_, [(W2, 4)] + CN),
                    mkap(xho[:], b0_, [(2, 4)] + CN), TT.add)
                nc.vector.tensor_tensor(
                    mkap(mh[:], b0_, [(W2, 2)] + CN),
                    mkap(qq[:], b0_, [(2 * W2, 2)] + CN),
                    mkap(t2[:], b0_, [(0, 2)] + CN), TT.mult)
                nc.vector.tensor_tensor(
                    mkap(hh[:], b0_, [(W2, 2)] + CN),
                    mkap(mh[:], b0_, [(W2, 2)] + CN),
                    mkap(qq[:], W2 + b0_, [(2 * W2, 2)] + CN), TT.add)
                nc.vector.tensor_tensor(
                    mkap(m1[:], b0_, CN), mkap(hh[:], b0_, CN),
                    mkap(t4[:], b0_, CN), TT.mult)
                nc.vector.tensor_tensor(
                    mkap(nh[:], b0_, CN), mkap(m1[:], b0_, CN),
                    mkap(hh[:], W2 + b0_, CN), TT.add)

            d = opool.tile([P, W2], f16, tag="d")
            ld = opool.tile([P, W2], f16, tag="ld")
            r = opool.tile([P, W2], f16, tag="r")
            oh = opool.tile([P, W2], f16, tag="oh")

            def div_out(ranges):
                # denominator (custom DVE op, no cancellation near tau=1);
                # 1/d via exp(-ln d) on ACT (builtin Reciprocal is blocked);
                # final mult on DVE, then one DMA out. All denoms are emitted
                # first so DVE stays busy while ACT runs ln/exp (tail path).
                for n0, nw in ranges:
                    sub = [(S, NFB), (1, nw)]
                    nc.vector._custom_dve(DENOM_OP, out=mkap(d[:], n0, sub),
                                          in0=mkap(tau[:], n0, sub))
                for n0, nw in ranges:
                    sub = [(S, NFB), (1, nw)]
                    nl = min(nw, L - n0)  # clip the junk column from the store
                    nc.scalar.activation(mkap(ld[:], n0, sub),
                                         mkap(d[:], n0, sub), LN)
                    nc.scalar.activation(mkap(r[:], n0, sub),
                                         mkap(ld[:], n0, sub), EXP, scale=-1.0)
                    nc.vector.tensor_tensor(
                        mkap(oh[:], n0, sub), mkap(nh[:], n0, sub),
                        mkap(r[:], n0, sub), TT.mult)
                    nc.sync.dma_start(
                        bass.AP(yt_d.tensor, b * NFB * P * L + n0,
                                [[L, P], [P * L, NFB], [1, nl]]),
                        mkap(oh[:], n0, [(S, NFB), (1, nl)]))

            if b == 0:
                # ramp the PE clock while the first DMAs land
                for _ in range(4):
                    nc.tensor.matmul(pss[(0, 0)][:], warm[:, :P],
                                     warm[:], start=True, stop=True)
            if b <= 2:
                # fill-latency path: run the conv one l-half at a time so
                # the tree starts after only half of each early conv; keeps
                # DVE fed until PE gets far enough ahead. For b0 the very
                # first half-tree is additionally split per feature block,
                # so DVE starts after only 16 of 64 conv matmuls.
                conv_half(0, 0)
                sigmoid_half(0, 0)
                if b == 0:
                    for c in range(NFB):
                        squares(0, LT, c, 1)
                        tree(0, LT, c, 1)
                else:
                    squares(0, LT)
                    tree(0, LT)
                conv_half(1, L - LT)
                sigmoid_half(1, L - LT)
                squares(LT, LE - LT)
                tree(LT, LE - LT)
                div_out([(0, LE)])
            elif b == BPC - 1:
                # tail path: denominators BEFORE the tree so ACT's ln/exp
                # overlap the tree ops; only the two oh mults + DMA remain
                # after the last tree op.
                conv_both()
                sigmoid_half(0, 0)
                sigmoid_half(1, L - LT)
                squares(0, LE)
                for n0, nw in ((0, LT), (LT, LE - LT)):
                    sub = [(S, NFB), (1, nw)]
                    nc.vector._custom_dve(DENOM_OP, out=mkap(d[:], n0, sub),
                                          in0=mkap(tau[:], n0, sub))
                    nc.scalar.activation(mkap(ld[:], n0, sub),
                                         mkap(d[:], n0, sub), LN)
                    nc.scalar.activation(mkap(r[:], n0, sub),
                                         mkap(ld[:], n0, sub), EXP,
                                         scale=-1.0)
                tree(0, LE)
                for n0, nw in ((0, LT), (LT, LE - LT)):
                    sub = [(S, NFB), (1, nw)]
                    nl = min(nw, L - n0)
                    nc.vector.tensor_tensor(
                        mkap(oh[:], n0, sub), mkap(nh[:], n0, sub),
                        mkap(r[:], n0, sub), TT.mult)
                    for ob in range(NFB):  # per-FB chunks: parallel queues
                        nc.sync.dma_start(
                            bass.AP(yt_d.tensor,
                                    (b * NFB + ob) * P * L + n0,
                                    [[L, P], [1, nl]]),
                            mkap(oh[:], ob * S + n0, [(1, nl)]))
            else:
                conv_both()
                sigmoid_half(0, 0)
                sigmoid_half(1, L - LT)
                squares(0, LE)
                tree(0, LE)
                div_out([(0, LE)])

    nc.compile()
    return nc


_NC = None


def _get_nc():
    global _NC
    if _NC is None:
        _NC = build_module()
    return _NC


def prep_inputs(x, conv_w, conv_b):
    xt = np.ascontiguousarray(
        x.transpose(0, 2, 1)).astype(np.float16)
    xt = xt.reshape(B, NFB, P, S)
    # one-seq-element-shifted copy (last column duplicates, never read)
    xo = np.concatenate([xt[..., 1:], xt[..., -1:]], axis=-1)
    xo = np.ascontiguousarray(xo)
    # [P, (k ic f)]: partition-contiguous rows for a single fast DMA
    wt = np.ascontiguousarray(
        conv_w.transpose(2, 1, 0)).astype(np.float16)
    wt = wt.reshape(K, NFB, P, F).transpose(2, 0, 1, 3)
    wt = np.ascontiguousarray(wt).reshape(P, K * NFB * F)
    cb = np.ascontiguousarray(conv_b, dtype=np.float32).reshape(F, 1)
    return xt, xo, wt, cb


def make_in_maps(x, conv_w, conv_b):
    xt, xo, wt, cb = prep_inputs(x, conv_w, conv_b)
    return [
        {"xh": xt[c * BPC:(c + 1) * BPC], "xo": xo[c * BPC:(c + 1) * BPC],
         "wt": wt, "cb": cb}
        for c in range(NCORES)
    ]


def gather_output(results):
    out = np.empty((B, L, F), np.float32)
    for c in range(NCORES):
        yt = results[c]["yt"].astype(np.float32)  # [BPC, NFB, P, L]
        out[c * BPC:(c + 1) * BPC] = (
            yt.transpose(0, 3, 1, 2).reshape(BPC, L, F))
    return out


def kernel(x, conv_w, conv_b):
    nc = _get_nc()
    in_maps = make_in_maps(x, conv_w, conv_b)
    res = run_bass_kernel_spmd(nc, in_maps, core_ids=list(range(NCORES)))
    return gather_output(res.results)


# revision 28
# speedup vs baseline: 1.0028x; 1.0028x over previous
"""Trainium2 Bass kernel for nn_ConvFilter (geometric-series conv filter).

Math (per batch b, output position l, feature f):
    t[o,l]  = sum_{i,k} conv_w[o,i,k] * x[l+k,i]          (valid conv, L=S-K+1)
    tau     = sigmoid(t + bias)
    out     = (sum_i tau^(7-i) * x[l+i,f]) / (sum_i tau^i)

v2 design (bottleneck was DVE at 75% busy):
  * x converted to fp16 on host; two dram copies (x16 and x16 shifted by one
    seq element) are DMA'd straight to SBUF -> no on-device fp32->fp16 ACT
    passes, and the conv matmuls run on the same fp16 tiles (full-rate PE).
  * numerator in fp16 on DVE (2x packed mode):
        q_j = tau*x_{2j} + x_{2j+1}
        N   = (q0*T2 + q1)*T4 + (q2*T2 + q3),   T2 = tau^2, T4 = tau^4
    odd-shift windows read the one-element-shifted x copy so every window
    stays 4-byte aligned (keeps the DVE 2x mode).
  * denominator D = (1+tau)(1+tau^2)(1+tau^4) as one custom DVE op (fp16 out);
    reciprocal moved to the ACT engine (builtin Reciprocal table);
    out = N * rho in fp16, DMA'd out as fp16, host converts to fp32.
  * engine split: ACT does sigmoid/squares/reciprocal, DVE the 14-op tree +
    denom + final mult, PE only matmuls.
  * data-parallel over batch: 8 batches/core on 8 cores, weights replicated.
"""

import numpy as np
from contextlib import ExitStack

import concourse.bass as bass
import concourse.tile as tile
from concourse import bacc, mybir
from concourse.bass_utils import run_bass_kernel_spmd
from concourse import dve_ops
from concourse.dve_ops import DveOp
from concourse.dve_spec import Spec, Src0, Src1, lower, sq, One, _has_src1
from concourse.dve_uop import DveOpSpec

B, S, F, K = 64, 1024, 256, 8
L = S - K + 1  # 1017
NCORES = 8
BPC = B // NCORES
P = 128
NFB = F // P  # 2 feature blocks
LT = 512      # matmul l-tile width (one PSUM bank)
LE = L + 1    # even fp16 elementwise width (DVE 2x mode needs even counts)


def _register_op(name, spec, subdim=False):
    for existing in dve_ops.OPS:
        if existing.name == name:
            return existing
    shas = {}
    for ver in ("v3", "v4"):
        tmp = DveOpSpec(name=name, opcode=0, uops=lower(spec, ver=ver),
                        rd1_en=_has_src1(spec))
        shas[ver] = tmp.sha(ver)
    op = DveOp(name, spec, subdim=subdim, uops_sha=shas)
    dve_ops.OPS.append(op)
    dve_ops.CUSTOM_DVE_SPECS[name] = spec
    dve_ops._SUB_OPCODE_FOR_NAME[name] = (
        dve_ops._CUSTOM_DVE_ROW_BASE + len(dve_ops.OPS) - 1
    )
    assert dve_ops._SUB_OPCODE_FOR_NAME[name] < 0x20
    return op


def _get_ops():
    _t2 = sq(Src0)
    _t4 = sq(_t2)
    denom_spec = Spec(
        body=(Src0 + One) * (_t2 + One) * (_t4 + One),
        reference=lambda in0, in1, s0, s1, imm2: (
            (1.0 + in0) * (1.0 + in0 * in0) * (1.0 + in0 ** 4)
        ).astype(np.float32),
    )
    return _register_op("ANT_CF_DENOM", denom_spec)


def build_module():
    DENOM_OP = _get_ops()
    f32 = mybir.dt.float32
    f16 = mybir.dt.float16
    TT = mybir.AluOpType
    SIG = mybir.ActivationFunctionType.Sigmoid
    SQU = mybir.ActivationFunctionType.Square
    LN = mybir.ActivationFunctionType.Ln
    EXP = mybir.ActivationFunctionType.Exp

    nc = bacc.Bacc("TRN2", target_bir_lowering=False, debug=False,
                   enable_asserts=False, num_devices=NCORES)

    xh_d = nc.dram_tensor("xh", [BPC, NFB, P, S], f16, kind="ExternalInput").ap()
    xo_d = nc.dram_tensor("xo", [BPC, NFB, P, S], f16, kind="ExternalInput").ap()
    wt_d = nc.dram_tensor("wt", [P, K * NFB * F], f16, kind="ExternalInput").ap()
    cb_d = nc.dram_tensor("cb", [F, 1], f32, kind="ExternalInput").ap()
    yt_d = nc.dram_tensor("yt", [BPC, NFB, P, L], f16, kind="ExternalOutput").ap()

    def mkap(base, off, dims):
        # raw AP: keep the tile's partition dim, replace free dims;
        # dims = [(stride, size), ...] in elements
        return bass.AP(base.tensor, base.offset + off,
                       [base.ap[0]] + [[s, n] for s, n in dims])

    with tile.TileContext(nc) as tc, ExitStack() as ctx:
        wpool = ctx.enter_context(tc.tile_pool(name="w", bufs=1))
        xpool = ctx.enter_context(tc.tile_pool(name="x", bufs=2))
        tpool = ctx.enter_context(tc.tile_pool(name="t", bufs=3))
        qpool = ctx.enter_context(tc.tile_pool(name="q", bufs=1))
        opool = ctx.enter_context(tc.tile_pool(name="o", bufs=2))
        ppool = ctx.enter_context(tc.tile_pool(name="p", bufs=2, space="PSUM"))

        # weights + bias: loaded once; host pre-packs the [p, (k ic f)]
        # layout, and the load is split into 4 DMAs so it spreads across
        # hardware queues (one instruction rides a single ~22GB/s engine)
        w_all = wpool.tile([P, K * NFB * F], f16, tag="wall")
        WQ = K * NFB * F // 4
        for c in range(4):
            nc.sync.dma_start(w_all[:, c * WQ:(c + 1) * WQ],
                              wt_d[:, c * WQ:(c + 1) * WQ])

        def wslice(k, ic, ob):
            base = (k * NFB + ic) * F + ob * P
            return w_all[:, base:base + P]

        bias_sb = wpool.tile([P, NFB], f32, tag="bias")
        nc.sync.dma_start(
            bias_sb[:], cb_d.rearrange("(ob p) one -> p (ob one)", p=P))

        W2 = NFB * S

        # warm-up: ramp the PE clock while the weight/x DMAs land; reads a
        # never-written scratch tile (values irrelevant), results overwritten
        # by batch 0's first accumulation (start=True).
        warm = wpool.tile([P, LT], f16, tag="warm")
        nc.gpsimd.memset(warm[:], 0.0)

        for b in range(BPC):
            # fp16 x (even-aligned) and its one-element-shifted copy, both
            # feature blocks side by side: [128, 2048] fp16; one DMA each
            xh = xpool.tile([P, W2], f16, tag="xh")
            xho = xpool.tile([P, W2], f16, tag="xho")
            # b0: also split along seq at col 520 (= LT + K) so the first
            # conv half + tree half only wait for the first chunk
            chunks = ((0, LT + K), (LT + K, S)) if b == 0 else ((0, S),)
            for src_d, dst in ((xh_d, xh), (xo_d, xho)):
                for ic in range(NFB):  # per-FB: parallel queues
                    for c0, c1 in chunks:
                        nc.sync.dma_start(
                            dst[:, ic * S + c0: ic * S + c1],
                            bass.AP(src_d.tensor,
                                    (b * NFB + ic) * P * S + c0,
                                    [[S, P], [1, c1 - c0]]))

            pss = {}
            for ob in range(NFB):
                for li, l0 in enumerate((0, L - LT)):
                    pss[(ob, li)] = ppool.tile([P, LT], f32, tag=f"ps{ob}{li}",
                                               name=f"ps{ob}{li}_{b}")

            def conv_half(li, l0):
                for ic in range(NFB):
                    for k in range(K):
                        first = (ic == 0 and k == 0)
                        last = (ic == NFB - 1 and k == K - 1)
                        for ob in range(NFB):
                            nc.tensor.matmul(
                                pss[(ob, li)][:],
                                wslice(k, ic, ob),
                                xh[:, ic * S + l0 + k: ic * S + l0 + k + LT],
                                start=first, stop=last,
                            )

            def conv_both():
                # weight-major so each LDWEIGHTS serves both l-tiles
                for ic in range(NFB):
                    for k in range(K):
                        first = (ic == 0 and k == 0)
                        last = (ic == NFB - 1 and k == K - 1)
                        for ob in range(NFB):
                            for li, l0 in enumerate((0, L - LT)):
                                nc.tensor.matmul(
                                    pss[(ob, li)][:],
                                    wslice(k, ic, ob),
                                    xh[:, ic * S + l0 + k:
                                       ic * S + l0 + k + LT],
                                    start=first, stop=last,
                                )

            tau = tpool.tile([P, W2], f16, tag="tau")
            t2 = tpool.tile([P, W2], f16, tag="t2")
            t4 = tpool.tile([P, W2], f16, tag="t4")
            uu = qpool.tile([P, 4 * W2], f16, tag="uu")
            qq = qpool.tile([P, 4 * W2], f16, tag="qq")
            mh = qpool.tile([P, 2 * W2], f16, tag="mh")
            hh = qpool.tile([P, 2 * W2], f16, tag="hh")
            m1 = qpool.tile([P, W2], f16, tag="m1")
            nh = qpool.tile([P, W2], f16, tag="nh")

            def sigmoid_half(li, l0):
                for ob in range(NFB):
                    nc.scalar.activation(
                        tau[:, ob * S + l0: ob * S + l0 + LT],
                        pss[(ob, li)][:], SIG,
                        bias=bias_sb[:, ob:ob + 1], scale=1.0)

            def squares(n0, nw):
                sub = [(S, NFB), (1, nw)]
                nc.scalar.activation(mkap(t2[:], n0, sub),
                                     mkap(tau[:], n0, sub), SQU)
                nc.scalar.activation(mkap(t4[:], n0, sub),
                                     mkap(t2[:], n0, sub), SQU)

            def tree(n0, nw):
                # Merged numerator tree: independent ops fused along an extra
                # j dim (stride-2 windows on x, stride-0 broadcast on tau/t2).
                # u_j = tau * x_{2j}            j=0..3   (one op)
                # q_j = u_j + x_{2j+1}          j=0..3   (one op)
                # (m0, h1) = (q0, q2) * t2               (one op)
                # (h0, h2) = (m0, h1) + (q1, q3)         (one op)
                # m1 = h0 * t4 ; nh = m1 + h2            (two ops)
                CN = [(S, NFB), (1, nw)]
                nc.vector.tensor_tensor(
                    mkap(uu[:], n0, [(W2, 4)] + CN),
                    mkap(tau[:], n0, [(0, 4)] + CN),
                    mkap(xh[:], n0, [(2, 4)] + CN), TT.mult)
                nc.vector.tensor_tensor(
                    mkap(qq[:], n0, [(W2, 4)] + CN),
                    mkap(uu[:], n0, [(W2, 4)] + CN),
                    mkap(xho[:], n0, [(2, 4)] + CN), TT.add)
                nc.vector.tensor_tensor(
                    mkap(mh[:], n0, [(W2, 2)] + CN),
                    mkap(qq[:], n0, [(2 * W2, 2)] + CN),
                    mkap(t2[:], n0, [(0, 2)] + CN), TT.mult)
                nc.vector.tensor_tensor(
                    mkap(hh[:], n0, [(W2, 2)] + CN),
                    mkap(mh[:], n0, [(W2, 2)] + CN),
                    mkap(qq[:], W2 + n0, [(2 * W2, 2)] + CN), TT.add)
                nc.vector.tensor_tensor(
                    mkap(m1[:], n0, CN), mkap(hh[:], n0, CN),
                    mkap(t4[:], n0, CN), TT.mult)
                nc.vector.tensor_tensor(
                    mkap(nh[:], n0, CN), mkap(m1[:], n0, CN),
                    mkap(hh[:], W2 + n0, CN), TT.add)

            d = opool.tile([P, W2], f16, tag="d")
            ld = opool.tile([P, W2], f16, tag="ld")
            r = opool.tile([P, W2], f16, tag="r")
            oh = opool.tile([P, W2], f16, tag="oh")

            def div_out(ranges):
                # denominator (custom DVE op, no cancellation near tau=1);
                # 1/d via exp(-ln d) on ACT (builtin Reciprocal is blocked);
                # final mult on DVE, then one DMA out. All denoms are emitted
                # first so DVE stays busy while ACT runs ln/exp (tail path).
                for n0, nw in ranges:
                    sub = [(S, NFB), (1, nw)]
                    nc.vector._custom_dve(DENOM_OP, out=mkap(d[:], n0, sub),
                                          in0=mkap(tau[:], n0, sub))
                for n0, nw in ranges:
                    sub = [(S, NFB), (1, nw)]
                    nl = min(nw, L - n0)  # clip the junk column from the store
                    nc.scalar.activation(mkap(ld[:], n0, sub),
                                         mkap(d[:], n0, sub), LN)
                    nc.scalar.activation(mkap(r[:], n0, sub),
                                         mkap(ld[:], n0, sub), EXP, scale=-1.0)
                    nc.vector.tensor_tensor(
                        mkap(oh[:], n0, sub), mkap(nh[:], n0, sub),
                        mkap(r[:], n0, sub), TT.mult)
                    nc.sync.dma_start(
                        bass.AP(yt_d.tensor, b * NFB * P * L + n0,
                                [[L, P], [P * L, NFB], [1, nl]]),
                        mkap(oh[:], n0, [(S, NFB), (1, nl)]))

            if b == 0:
                # ramp the PE clock while the first DMAs land
                for _ in range(6):
                    nc.tensor.matmul(pss[(0, 0)][:], warm[:, :P],
                                     warm[:], start=True, stop=True)
            if b <= 2:
                # fill-latency path: run the conv one l-half at a time so
                # the tree starts after only half of each early conv; keeps
                # DVE fed until PE gets far enough ahead.
                conv_half(0, 0)
                sigmoid_half(0, 0)
                squares(0, LT)
                tree(0, LT)
                conv_half(1, L - LT)
                sigmoid_half(1, L - LT)
                squares(LT, LE - LT)
                tree(LT, LE - LT)
                div_out([(0, LE)])
            elif b == BPC - 1:
                # tail path: denominators BEFORE the tree so ACT's ln/exp
                # overlap the tree ops; only the two oh mults + DMA remain
                # after the last tree op.
                conv_both()
                sigmoid_half(0, 0)
                sigmoid_half(1, L - LT)
                squares(0, LE)
                for n0, nw in ((0, LT), (LT, LE - LT)):
                    sub = [(S, NFB), (1, nw)]
                    nc.vector._custom_dve(DENOM_OP, out=mkap(d[:], n0, sub),
                                          in0=mkap(tau[:], n0, sub))
                    nc.scalar.activation(mkap(ld[:], n0, sub),
                                         mkap(d[:], n0, sub), LN)
                    nc.scalar.activation(mkap(r[:], n0, sub),
                                         mkap(ld[:], n0, sub), EXP,
                                         scale=-1.0)
                tree(0, LE)
                for n0, nw in ((0, LT), (LT, LE - LT)):
                    sub = [(S, NFB), (1, nw)]
                    nl = min(nw, L - n0)
                    nc.vector.tensor_tensor(
                        mkap(oh[:], n0, sub), mkap(nh[:], n0, sub),
                        mkap(r[:], n0, sub), TT.mult)
                    # quarter chunks across queues to shrink the end drain
                    for ob in range(NFB):
                        for s0, s1 in ((0, nl // 2), (nl // 2, nl)):
                            nc.sync.dma_start(
                                bass.AP(yt_d.tensor,
                                        (b * NFB + ob) * P * L + n0 + s0,
                                        [[L, P], [1, s1 - s0]]),
                                mkap(oh[:], ob * S + n0 + s0,
                                     [(1, s1 - s0)]))
            else:
                conv_both()
                sigmoid_half(0, 0)
                sigmoid_half(1, L - LT)
                squares(0, LE)
                tree(0, LE)
                div_out([(0, LE)])

    nc.compile()
    return nc


_NC = None


def _get_nc():
    global _NC
    if _NC is None:
        _NC = build_module()
    return _NC


def prep_inputs(x, conv_w, conv_b):
    xt = np.ascontiguousarray(
        x.transpose(0, 2, 1)).astype(np.float16)
    xt = xt.reshape(B, NFB, P, S)
    # one-seq-element-shifted copy (last column duplicates, never read)
    xo = np.concatenate([xt[..., 1:], xt[..., -1:]], axis=-1)
    xo = np.ascontiguousarray(xo)
    # [P, (k ic f)]: partition-contiguous rows for a single fast DMA
    wt = np.ascontiguousarray(
        conv_w.transpose(2, 1, 0)).astype(np.float16)
    wt = wt.reshape(K, NFB, P, F).transpose(2, 0, 1, 3)
    wt = np.ascontiguousarray(wt).reshape(P, K * NFB * F)
    cb = np.ascontiguousarray(conv_b, dtype=np.float32).reshape(F, 1)
    return xt, xo, wt, cb


def make_in_maps(x, conv_w, conv_b):
    xt, xo, wt, cb = prep_inputs(x, conv_w, conv_b)
    return [
        {"xh": xt[c * BPC:(c + 1) * BPC], "xo": xo[c * BPC:(c + 1) * BPC],
         "wt": wt, "cb": cb}
        for c in range(NCORES)
    ]


def gather_output(results):
    out = np.empty((B, L, F), np.float32)
    for c in range(NCORES):
        yt = results[c]["yt"].astype(np.float32)  # [BPC, NFB, P, L]
        out[c * BPC:(c + 1) * BPC] = (
            yt.transpose(0, 3, 1, 2).reshape(BPC, L, F))
    return out


def kernel(x, conv_w, conv_b):
    nc = _get_nc()
    in_maps = make_in_maps(x, conv_w, conv_b)
    res = run_bass_kernel_spmd(nc, in_maps, core_ids=list(range(NCORES)))
    return gather_output(res.results)
